# revision 1
# baseline (speedup 1.0000x reference)
# PointNet++ feature-propagation (three_nn + three_interpolate + shared MLP)
# Trainium2 Bass/Tile kernel, 8 NeuronCores, data-parallel over batch.
#
# Per batch (n=4096 unknown, m=1024 known, C2=512, C1=256):
#  1) coarse: D[i,j] = 2*u_i.k_j - |k_j|^2 via ONE bf16 matmul with K=21 rows
#     (triple bf16 splits) -> top-8 candidates per point (nc.vector.max/max_index)
#  2) refine: gather candidate coords (DGE gather from padded HBM table),
#     recompute d2 = ((ux-kx)^2 + (uy-ky)^2) + (uz-kz)^2 in fp32 exactly as the
#     reference does, re-rank top-3 of 8 with j-ascending tie-break
#     -> bit-exact selection + exact inverse-distance weights
#  3) features: DGE transpose-gather of bf16 feats_t rows (lands channel-major),
#     weighted-sum with PE-broadcast weight tiles
#  4) MLP: bf16 matmuls, fp32 PSUM accumulation, relu on ACT, fp32 output
import numpy as np
from contextlib import ExitStack

import concourse.bass as bass
import concourse.bacc as bacc
import concourse.tile as tile
import concourse.mybir as mybir
from concourse.masks import make_identity

AP = bass.AP
dt = mybir.dt
Alu = mybir.AluOpType
ACTF = mybir.ActivationFunctionType

B_FULL = 16
N_CORES = 8
NB = 2            # batches per core
N = 4096
M = 1024
C1 = 256
C2 = 512
D0 = 256
D1 = 256
EPS = 1e-8

NCH = N // 128    # 32 i-chunks
MCH = M // 128    # 8 j-chunks
HALF = N // 2     # 2048 (i-split for SBUF residency)
HCH = NCH // 2    # 16 chunks per half
QCH = 8           # chunks per refine-quarter
KROWS = 21
KTE = 64          # ktab row elements (fp32, 256B DGE minimum)


def _bf16_split3(nc, pool, x_ap, shape):
    """bf16 (hi, lo, mid) with hi+lo+mid ~= x."""
    xh = pool.tile(list(shape), dt.bfloat16, tag="sp_h")
    xl = pool.tile(list(shape), dt.bfloat16, tag="sp_l")
    xm = pool.tile(list(shape), dt.bfloat16, tag="sp_m")
    r1 = pool.tile(list(shape), dt.float32, tag="sp_r1")
    r2 = pool.tile(list(shape), dt.float32, tag="sp_r2")
    nc.vector.tensor_copy(xh[:], x_ap)
    nc.vector.tensor_sub(r1[:], x_ap, xh[:])
    nc.vector.tensor_copy(xl[:], r1[:])
    nc.vector.tensor_sub(r2[:], r1[:], xl[:])
    nc.vector.tensor_copy(xm[:], r2[:])
    return xh, xl, xm


def _v(t_ap, dims, off=0):
    """AP over t_ap's tensor with explicit [stride, count] dims (dims[0] = partition dim)."""
    return AP(t_ap.tensor, t_ap.offset + off, dims)


def build_nc(nb=NB):
    nc = bacc.Bacc("TRN2", target_bir_lowering=False, debug=False)

    unknown_h = nc.dram_tensor("unknown", [nb, N, 3], dt.float32, kind="ExternalInput")
    known_h = nc.dram_tensor("known", [nb, M, 3], dt.float32, kind="ExternalInput")
    uf_h = nc.dram_tensor("unknow_feats", [nb, C1, N], dt.float32, kind="ExternalInput")
    kf_h = nc.dram_tensor("known_feats", [nb, C2, M], dt.float32, kind="ExternalInput")
    w0_h = nc.dram_tensor("W0", [C1 + C2, D0], dt.float32, kind="ExternalInput")
    w1_h = nc.dram_tensor("W1", [D0, D1], dt.float32, kind="ExternalInput")
    out_h = nc.dram_tensor("out", [nb, D1, N], dt.float32, kind="ExternalOutput")

    ft_h = [nc.dram_tensor(f"ft{b}", [M, C2], dt.bfloat16) for b in range(nb)]
    kt_h = [nc.dram_tensor(f"ktab{b}", [M, KTE], dt.float32) for b in range(nb)]

    with tile.TileContext(nc) as tc, ExitStack() as ctx:
        const = ctx.enter_context(tc.tile_pool(name="const", bufs=1))
        prep = ctx.enter_context(tc.tile_pool(name="prep", bufs=1))
        sp = ctx.enter_context(tc.tile_pool(name="split", bufs=2))
        dsbp = ctx.enter_context(tc.tile_pool(name="dsbp", bufs=3))
        sel = ctx.enter_context(tc.tile_pool(name="sel", bufs=2))
        ref = ctx.enter_context(tc.tile_pool(name="refine", bufs=1))
        refq = ctx.enter_context(tc.tile_pool(name="refq", bufs=2))
        kcp = ctx.enter_context(tc.tile_pool(name="kcp", bufs=1))
        gat = ctx.enter_context(tc.tile_pool(name="gat", bufs=2))
        wts = ctx.enter_context(tc.tile_pool(name="wts", bufs=1))
        wtmp = ctx.enter_context(tc.tile_pool(name="wtmp", bufs=2))
        gwp = ctx.enter_context(tc.tile_pool(name="gwp", bufs=1))
        mlpp = ctx.enter_context(tc.tile_pool(name="mlpp", bufs=1))
        ftp = ctx.enter_context(tc.tile_pool(name="ftp", bufs=1))
        ps_d = ctx.enter_context(tc.tile_pool(name="ps_d", bufs=3, space="PSUM"))
        ps_mm = ctx.enter_context(tc.tile_pool(name="ps_mm", bufs=2, space="PSUM"))
        ps_tr = ctx.enter_context(tc.tile_pool(name="ps_tr", bufs=2, space="PSUM"))

        # ---------------- constants ----------------
        ident_b = const.tile([128, 128], dt.bfloat16, tag="idb")
        make_identity(nc, ident_b[:])
        ident_u = const.tile([128, 128], dt.float16, tag="idu")
        make_identity(nc, ident_u[:])
        ones_b = const.tile([1, 128], dt.bfloat16, tag="ones")
        nc.vector.memset(ones_b[:], 1.0)

        w0_sb = const.tile([128, 6, D0], dt.bfloat16, tag="w0")
        w1_sb = const.tile([128, 2, D1], dt.bfloat16, tag="w1")
        w0_f = const.tile([128, 6, D0], dt.float32, tag="w0f")
        w1_f = const.tile([128, 2, D1], dt.float32, tag="w1f")
        for ci in range(6):
            nc.sync.dma_start(w0_f[:, ci, :], w0_h.ap()[128 * ci:128 * ci + 128, :])
            nc.vector.tensor_copy(w0_sb[:, ci, :], w0_f[:, ci, :])
        for ci in range(2):
            nc.sync.dma_start(w1_f[:, ci, :], w1_h.ap()[128 * ci:128 * ci + 128, :])
            nc.vector.tensor_copy(w1_sb[:, ci, :], w1_f[:, ci, :])

        for b in range(nb):
            # ======== stage known_feats^T as bf16 rows in HBM ========
            kf16 = ftp.tile([128, 4, M], dt.bfloat16, tag="kf16")
            for cj in range(4):
                kf32 = ftp.tile([128, M], dt.float32, tag="kf32")
                nc.sync.dma_start(kf32[:], kf_h.ap()[b, 128 * cj:128 * cj + 128, :])
                nc.vector.tensor_copy(kf16[:, cj, :], kf32[:])
            ftsb = ftp.tile([128, MCH, C2], dt.bfloat16, tag="ftsb")
            for mt in range(MCH):
                for cj in range(4):
                    pst = ps_tr.tile([128, 128], dt.bfloat16, tag="tr")
                    nc.tensor.transpose(
                        pst[:], kf16[:, cj, 128 * mt:128 * mt + 128], ident_b[:]
                    )
                    nc.scalar.copy(ftsb[:, mt, 128 * cj:128 * cj + 128], pst[:])
            nc.sync.dma_start(
                _v(ft_h[b].ap(), [[C2, 128], [128 * C2, MCH], [1, C2]]),
                ftsb[:],
            )

            # ======== known prep ========
            kw = prep.tile([128, MCH, 3], dt.float32, tag="kw")
            nc.sync.dma_start(
                kw[:], AP(known_h, b * M * 3, [[3, 128], [3 * 128, MCH], [1, 3]])
            )
            # ktab rows [kx, ky, kz, 0...] fp32 (for the refine coord-gather)
            kt_sb = prep.tile([128, MCH, KTE], dt.float32, tag="kt_sb")
            nc.vector.memset(kt_sb[:], 0.0)
            nc.vector.tensor_copy(kt_sb[:, :, 0:3], kw[:])
            nc.sync.dma_start(
                _v(kt_h[b].ap(), [[KTE, 128], [128 * KTE, MCH], [1, KTE]]),
                kt_sb[:],
            )

            k2 = prep.tile([128, MCH, 3], dt.float32, tag="k2")
            nc.vector.tensor_scalar_mul(k2[:], kw[:], 2.0)
            k2h, k2l, k2m = _bf16_split3(nc, sp, k2[:], [128, MCH, 3])
            sq = prep.tile([128, MCH, 3], dt.float32, tag="ksq")
            nc.scalar.square(sq[:], kw[:])
            s_f = prep.tile([128, MCH], dt.float32, tag="ks")
            nc.vector.tensor_add(s_f[:], sq[:, :, 0], sq[:, :, 1])
            nc.vector.tensor_add(s_f[:], s_f[:], sq[:, :, 2])
            ns = prep.tile([128, MCH], dt.float32, tag="kns")
            nc.vector.tensor_scalar_mul(ns[:], s_f[:], -1.0)
            nsh, nsl, nsm = _bf16_split3(nc, sp, ns[:], [128, MCH])

            # rows: 0-2 uh|2kh, 3-5 uh|2kl, 6-8 ul|2kh, 9-11 ul|2kl,
            #       12-14 uh|2km, 15-17 um|2kh, 18 1|-sh, 19 1|-sl, 20 1|-sm
            kch = prep.tile([128, MCH, 24], dt.bfloat16, tag="kch")
            for (r0, src) in ((0, k2h), (3, k2l), (6, k2h), (9, k2l), (12, k2m), (15, k2h)):
                nc.vector.tensor_copy(kch[:, :, r0:r0 + 3], src[:])
            nc.vector.tensor_copy(kch[:, :, 18], nsh[:])
            nc.vector.tensor_copy(kch[:, :, 19], nsl[:])
            nc.vector.tensor_copy(kch[:, :, 20], nsm[:])
            rhs_all = prep.tile([KROWS, M], dt.bfloat16, tag="rhs_all")
            for t in range(MCH):
                pst = ps_tr.tile([32, 128], dt.bfloat16, tag="tr")
                nc.tensor.transpose(pst[:KROWS, :], kch[:, t, :KROWS], ident_b[:])
                nc.scalar.copy(rhs_all[:, 128 * t:128 * t + 128], pst[:KROWS, :])

            # ======== unknown prep ========
            uw = prep.tile([128, NCH, 3], dt.float32, tag="uw")
            nc.sync.dma_start(
                uw[:], AP(unknown_h, b * N * 3, [[3, 128], [3 * 128, NCH], [1, 3]])
            )
            uh, ul, um = _bf16_split3(nc, sp, uw[:], [128, NCH, 3])
            uch = prep.tile([128, NCH, 24], dt.bfloat16, tag="uch")
            for (r0, src) in ((0, uh), (3, uh), (6, ul), (9, ul), (12, uh), (15, um)):
                nc.vector.tensor_copy(uch[:, :, r0:r0 + 3], src[:])
            nc.vector.memset(uch[:, :, 18:21], 1.0)
            lhs_all = prep.tile([KROWS, N], dt.bfloat16, tag="lhs_all")
            for t in range(NCH):
                pst = ps_tr.tile([32, 128], dt.bfloat16, tag="tr")
                nc.tensor.transpose(pst[:KROWS, :], uch[:, t, :KROWS], ident_b[:])
                nc.scalar.copy(lhs_all[:, 128 * t:128 * t + 128], pst[:KROWS, :])

            # ======== coarse: D matmul + top-8 per i-chunk ========
            vall = sel.tile([128, NCH, 8], dt.float32, tag="vall")
            miall = sel.tile([128, NCH, 8], dt.uint16, tag="miall")
            for t in range(NCH):
                dsb = dsbp.tile([128, M], dt.float32, tag="dsb")
                for hm in range(2):
                    psd = ps_d.tile([128, 512], dt.float32, tag="psd")
                    nc.tensor.matmul(
                        psd[:],
                        lhs_all[:, 128 * t:128 * t + 128],
                        rhs_all[:, 512 * hm:512 * hm + 512],
                        start=True,
                        stop=True,
                    )
                    nc.scalar.copy(dsb[:, 512 * hm:512 * hm + 512], psd[:])
                nc.vector.max(out=vall[:, t, :], in_=dsb[:])
                nc.vector.max_index(
                    out=miall[:, t, :], in_max=vall[:, t, :], in_values=dsb[:]
                )

            # ======== refine: exact d2 on the 8 candidates ========
            # sort candidate indices ascending (tie-break parity with lax.top_k)
            jf = ref.tile([128, NCH, 8], dt.float32, tag="jf")
            nc.vector.tensor_copy(jf[:], miall[:])
            nc.vector.tensor_scalar_mul(jf[:], jf[:], -1.0)
            sj = ref.tile([128, NCH, 8], dt.float32, tag="sj")
            for t in range(NCH):
                nc.vector.max(out=sj[:, t, :], in_=jf[:, t, :])
            js = ref.tile([128, NCH, 8], dt.float32, tag="js")
            nc.vector.tensor_scalar_mul(js[:], sj[:], -1.0)
            jsh = ref.tile([128, NCH, 8], dt.float16, tag="jsh")
            nc.vector.tensor_copy(jsh[:], js[:])

            d23all = ref.tile([128, NCH, 3], dt.float32, tag="d23all")
            j3h = ref.tile([128, NCH, 3], dt.float16, tag="j3h")
            for q in range(NCH // QCH):
                qsl = slice(QCH * q, QCH * q + QCH)
                # ---- wrapped idx layout for the coord gather (v=(t*8+c)*128+p)
                ps_t1 = ps_tr.tile([64, 128], dt.float16, tag="tr")
                nc.tensor.transpose(ps_t1[:], jsh[:, qsl, :], ident_u[:])
                mit8 = refq.tile([64, 128], dt.float16, tag="mit8")
                nc.vector.tensor_copy(mit8[:], ps_t1[:])
                idxw8 = refq.tile([128, QCH * 8 * 128 // 16], dt.int16, tag="idxw8")
                for s in range(8):
                    ps_t2 = ps_tr.tile([16, 64], dt.float16, tag="tr")
                    nc.tensor.transpose(
                        ps_t2[:], mit8[:, 16 * s:16 * s + 16], ident_u[:64, :64]
                    )
                    # out[q2, f=(t,c)] -> idxw8[q2, (t*8+c)*8 + s]
                    nc.vector.tensor_copy(
                        _v(idxw8[:16, :], [idxw8[:16, :].ap[0], [64, QCH], [8, 8]], off=s),
                        ps_t2[:],
                    )
                for gsz in (16, 32, 64):
                    nc.sync.dma_start(idxw8[gsz:2 * gsz, :], idxw8[0:gsz, :])
                kc = kcp.tile([128, QCH * 8, KTE], dt.float32, tag="kc")
                nc.gpsimd.dma_gather(
                    kc[:],
                    kt_h[b].ap(),
                    idxw8[:],
                    QCH * 8 * 128,
                    QCH * 8 * 128,
                    KTE,
                    transpose=False,
                    single_packet=False,
                )
                # ---- exact d2 (reference arithmetic): ((dx^2+dy^2)+dz^2)
                df = refq.tile([128, QCH, 8, 3], dt.float32, tag="df")
                nc.vector.tensor_sub(
                    df[:],
                    _v(kc[:], [kc[:].ap[0], [8 * KTE, QCH], [KTE, 8], [1, 3]]),
                    _v(uw[:], [uw[:].ap[0], [3, QCH], [0, 8], [1, 3]], off=3 * QCH * q),
                )
                sq2 = refq.tile([128, QCH, 8, 3], dt.float32, tag="sq2")
                nc.vector.tensor_mul(sq2[:], df[:], df[:])
                d2e = refq.tile([128, QCH, 8], dt.float32, tag="d2e")
                nc.vector.tensor_add(d2e[:], sq2[:, :, :, 0], sq2[:, :, :, 1])
                nc.vector.tensor_add(d2e[:], d2e[:], sq2[:, :, :, 2])
                nd2 = refq.tile([128, QCH, 8], dt.float32, tag="nd2")
                nc.vector.tensor_scalar_mul(nd2[:], d2e[:], -1.0)
                v8e = refq.tile([128, QCH, 8], dt.float32, tag="v8e")
                r8 = refq.tile([128, QCH, 8], dt.uint16, tag="r8")
                for t in range(QCH):
                    nc.vector.max(out=v8e[:, t, :], in_=nd2[:, t, :])
                    nc.vector.max_index(
                        out=r8[:, t, :], in_max=v8e[:, t, :], in_values=nd2[:, t, :]
                    )
                # d2_sel = -v8e[:3] + EPS  (exact, >= EPS > 0)
                nc.vector.tensor_scalar(
                    d23all[:, qsl, :], v8e[:, :, 0:3], -1.0, EPS, op0=Alu.mult, op1=Alu.add
                )
                # rank -> original index: j3 = sum_r (rank==r) * js[r]
                rf = refq.tile([128, QCH, 3], dt.float32, tag="rf")
                nc.vector.tensor_copy(rf[:], r8[:, :, 0:3])
                j3f = refq.tile([128, QCH, 3], dt.float32, tag="j3f")
                tmp3 = refq.tile([128, QCH, 3], dt.float32, tag="tmp3")
                for r in range(8):
                    dst = j3f if r == 0 else tmp3
                    nc.vector.scalar_tensor_tensor(
                        dst[:],
                        rf[:],
                        float(r),
                        _v(js[:], [js[:].ap[0], [8, QCH], [0, 3]], off=8 * QCH * q + r),
                        op0=Alu.is_equal,
                        op1=Alu.mult,
                    )
                    if r > 0:
                        nc.vector.tensor_add(j3f[:], j3f[:], tmp3[:])
                nc.vector.tensor_copy(j3h[:, qsl, :], j3f[:])

            # ======== per half: weights, gather+wsum, MLP ========
            for h in range(2):
                tsl = slice(HCH * h, HCH * h + HCH)
                # --- weights (fp32) from exact d2
                r3 = wts.tile([128, HCH, 3], dt.float32, tag="r3")
                nc.vector.reciprocal(r3[:], d23all[:, tsl, :])
                z = wts.tile([128, HCH], dt.float32, tag="z")
                nc.vector.tensor_reduce(z[:], r3[:], axis=mybir.AxisListType.X, op=Alu.add)
                iz = wts.tile([128, HCH], dt.float32, tag="iz")
                nc.vector.reciprocal(iz[:], z[:])
                w3b = wts.tile([128, HCH, 3], dt.bfloat16, tag="w3b")
                w3f = wts.tile([128, HCH, 3], dt.float32, tag="w3f")
                nc.vector.tensor_mul(w3f[:], r3[:], iz[:].to_broadcast([128, HCH, 3]))
                nc.vector.tensor_copy(w3b[:], w3f[:])

                interp = mlpp.tile([128, 4, HALF], dt.bfloat16, tag="interp")
                for k in range(3):
                    # --- wrow_k: (1, HALF) bf16 = weights of neighbor k in i-order
                    ps_tw = ps_tr.tile([16, 128], dt.bfloat16, tag="tr")
                    nc.tensor.transpose(ps_tw[:], w3b[:, :, k], ident_b[:])
                    wrow_sb = wtmp.tile([16, 128], dt.bfloat16, tag="wrow_sb")
                    nc.vector.tensor_copy(wrow_sb[:], ps_tw[:])
                    wrow = wtmp.tile([1, HALF], dt.bfloat16, tag="wrow")
                    nc.sync.dma_start(
                        _v(wrow[:], [wrow[:].ap[0], [128, HCH], [1, 128]]),
                        wrow_sb[:],
                    )
                    # --- idx wrap layout for neighbor k (v = t*128+p)
                    ps_ti = ps_tr.tile([16, 128], dt.float16, tag="tr")
                    nc.tensor.transpose(ps_ti[:], j3h[:, tsl, k], ident_u[:])
                    mit = wtmp.tile([16, 128], dt.float16, tag="mit")
                    nc.vector.tensor_copy(mit[:], ps_ti[:])
                    idxw = wtmp.tile([128, HALF // 16], dt.int16, tag="idxw")
                    for s in range(8):
                        ps_t2 = ps_tr.tile([16, 16], dt.float16, tag="tr")
                        nc.tensor.transpose(
                            ps_t2[:], mit[:, 16 * s:16 * s + 16], ident_u[:16, :16]
                        )
                        nc.vector.tensor_copy(
                            _v(idxw[:16, :], [idxw[:16, :].ap[0], [8, HCH]], off=s),
                            ps_t2[:],
                        )
                    for gsz in (16, 32, 64):
                        nc.sync.dma_start(idxw[gsz:2 * gsz, :], idxw[0:gsz, :])

                    # --- gather rows for neighbor k (channel-major bf16)
                    g_t = gat.tile([128, 4, HALF], dt.bfloat16, tag="g")
                    nc.gpsimd.dma_gather(
                        g_t[:],
                        ft_h[b].ap(),
                        idxw[:],
                        HALF,
                        HALF,
                        C2,
                        transpose=True,
                        single_packet=False,
                    )

                    # --- wb_k broadcast tile + multiply/accumulate
                    wb = wtmp.tile([128, HALF], dt.bfloat16, tag="wb")
                    for nci in range(HALF // 512):
                        ps_wb = ps_mm.tile([128, 512], dt.float32, tag="mm")
                        nc.tensor.matmul(
                            ps_wb[:],
                            ones_b[:],
                            wrow[0:1, 512 * nci:512 * nci + 512],
                            start=True,
                            stop=True,
                        )
                        nc.scalar.copy(wb[:, 512 * nci:512 * nci + 512], ps_wb[:])
                    wbb = _v(wb[:], [wb[:].ap[0], [0, 4], [1, HALF]])
                    if k == 0:
                        nc.vector.tensor_mul(interp[:], g_t[:], wbb)
                    else:
                        gw = gwp.tile([128, 4, HALF], dt.bfloat16, tag="gw")
                        nc.vector.tensor_mul(gw[:], g_t[:], wbb)
                        nc.vector.tensor_add(interp[:], interp[:], gw[:])

                # --- unknow_feats
                uf16 = mlpp.tile([128, 2, HALF], dt.bfloat16, tag="uf16")
                for cj in range(2):
                    uf32 = mlpp.tile([128, HALF], dt.float32, tag="uf32")
                    nc.sync.dma_start(
                        uf32[:],
                        uf_h.ap()[b, 128 * cj:128 * cj + 128, HALF * h:HALF * h + HALF],
                    )
                    nc.scalar.copy(uf16[:, cj, :], uf32[:])

                # --- MLP1 (relu) -> h_t bf16
                h_t = mlpp.tile([128, 2, HALF], dt.bfloat16, tag="h")
                for mj in range(2):
                    for nci in range(HALF // 512):
                        nsl_ = slice(512 * nci, 512 * nci + 512)
                        pm = ps_mm.tile([128, 512], dt.float32, tag="mm")
                        for ci in range(6):
                            rhs = interp[:, ci, nsl_] if ci < 4 else uf16[:, ci - 4, nsl_]
                            nc.tensor.matmul(
                                pm[:],
                                w0_sb[:, ci, 128 * mj:128 * mj + 128],
                                rhs,
                                start=(ci == 0),
                                stop=(ci == 5),
                            )
                        nc.scalar.activation(h_t[:, mj, nsl_], pm[:], ACTF.Relu, bias=0.0)

                # --- MLP2 (relu) -> fp32 out
                for mj in range(2):
                    o_t = mlpp.tile([128, HALF], dt.float32, tag="o")
                    for nci in range(HALF // 512):
                        nsl_ = slice(512 * nci, 512 * nci + 512)
                        pm = ps_mm.tile([128, 512], dt.float32, tag="mm")
                        for ci in range(2):
                            nc.tensor.matmul(
                                pm[:],
                                w1_sb[:, ci, 128 * mj:128 * mj + 128],
                                h_t[:, ci, nsl_],
                                start=(ci == 0),
                                stop=(ci == 1),
                            )
                        nc.scalar.activation(o_t[:, nsl_], pm[:], ACTF.Relu, bias=0.0)
                    nc.sync.dma_start(
                        out_h.ap()[b, 128 * mj:128 * mj + 128, HALF * h:HALF * h + HALF],
                        o_t[:],
                    )

    nc.compile()
    return nc


_NC_CACHE = {}


def _get_nc(nb=NB):
    if nb not in _NC_CACHE:
        _NC_CACHE[nb] = build_nc(nb)
    return _NC_CACHE[nb]


def kernel(**inputs):
    from concourse.bass_utils import run_bass_kernel_spmd

    nc = _get_nc(NB)
    per_core = B_FULL // N_CORES
    in_maps = []
    for c in range(N_CORES):
        sl = slice(per_core * c, per_core * (c + 1))
        in_maps.append(
            {
                "unknown": np.ascontiguousarray(np.asarray(inputs["unknown"][sl], dtype=np.float32)),
                "known": np.ascontiguousarray(np.asarray(inputs["known"][sl], dtype=np.float32)),
                "unknow_feats": np.ascontiguousarray(np.asarray(inputs["unknow_feats"][sl], dtype=np.float32)),
                "known_feats": np.ascontiguousarray(np.asarray(inputs["known_feats"][sl], dtype=np.float32)),
                "W0": np.asarray(inputs["W0"], dtype=np.float32),
                "W1": np.asarray(inputs["W1"], dtype=np.float32),
            }
        )
    res = run_bass_kernel_spmd(nc, in_maps, core_ids=list(range(N_CORES)))
    out = np.concatenate([res.results[c]["out"] for c in range(N_CORES)], axis=0)
    return out.astype(np.float32)

